# revision 2
# baseline (speedup 1.0000x reference)
"""Causal self-attention with RoPE (B=2, T=2048, C=1024, H=16, D=64) on 8
Trainium2 NeuronCores.

Sharding: tensor-parallel over heads — each core owns 2 heads (QKV and output
projections sliced on the head axis); the per-core partial outputs (full
[C, B*T] each) are summed on the host.

Per-core layout / algorithm (everything transposed: features on partitions,
tokens on the free axis):
  xT [1024, 4096]     x^T, shared by all cores
  QKV proj            qkv^T chunks via PE matmul (fp32r, 1 cyc/row),
                      W rows pre-permuted on host into three 128-row groups:
                        E = [q_h0_even(32) | q_h1_even | k_h0_even | k_h1_even]
                        O = same rows, odd dims
                        V = [v_h0(64) | v_h1(64)]
  RoPE                rotE = E*cos - O*sin, rotO = E*sin + O*cos on DVE,
                      written into qe/qo/ke/ko [64, 2048] (per-head 32-row
                      blocks, so scores contract e/o parts separately)
  scores^T            S^T[kj,qi] = ke^T·qe + ko^T·qo (two K=32 matmuls into
                      one PSUM bank; same row-group -> serialized, safe)
  softmax             exp on ScalarE (scale=1/8 folded in), causal triangle
                      via gpsimd affine_select, no max-subtraction needed
                      (|scores/8| <= ~8), column sums via 64 ones-columns
                      appended to v in the PV matmul
  PV                  y^T[d,qi] accumulated over kj chunks; v transposed
                      on-chip via PE transpose into v_all [128, 256*16]
  out proj            outT[c,t] partial = woutT · y^T, DMA'd out

Host gathers the 8 partial outT [1024, 4096] tensors, sums, transposes.
"""

import sys
import types

import numpy as np

import concourse.bass as bass
import concourse.tile as tile
from concourse import bacc
from concourse import mybir
from concourse.bass_utils import run_bass_kernel_spmd
from concourse.masks import make_identity

F32 = mybir.dt.float32
F32R = mybir.dt.float32r

B = 2
T = 2048
C = 1024
HEADS_PER_CORE = 2
D = 64
N_CORES = 8
BT = B * T              # 4096
TC = 512                # token chunk (free dim of most matmuls)
NTC = BT // TC          # 8 global token chunks, 4 per batch
NQI = T // TC           # 4 qi chunks per batch
NKJ = T // 128          # 16 kj chunks per batch
KC = C // 128           # 8 contraction chunks for the projections


def _install_ntff_hook():
    """bass_utils imports antenv.axon_hooks when tracing; this image lacks it.
    Recreate it from the ctypes NTFF driver so trace=True works."""
    if "antenv.axon_hooks" in sys.modules:
        return
    try:
        from trn_agent_boot.trn_boot import _ntff_profile_via_ctypes

        hook = _ntff_profile_via_ctypes("/opt/axon/libaxon_pjrt.so")
    except Exception:
        hook = None
    mod = types.ModuleType("antenv.axon_hooks")
    mod.get_axon_ntff_profile_hook = lambda: hook
    mod.set_axon_ntff_profile_hook = lambda h: None
    sys.modules["antenv.axon_hooks"] = mod


_install_ntff_hook()


def build_nc():
    nc = bacc.Bacc(None, target_bir_lowering=False, debug=False)

    xt = nc.declare_dram_parameter("xt", [C, BT], F32R, isOutput=False)
    wqkv = nc.declare_dram_parameter("wqkv", [128, KC * 384], F32R, isOutput=False)
    wout = nc.declare_dram_parameter("wout", [128, C], F32R, isOutput=False)
    cs = nc.declare_dram_parameter("cs", [128, 2 * T], F32, isOutput=False)
    ones = nc.declare_dram_parameter("ones", [128, 64], F32R, isOutput=False)
    outT = nc.declare_dram_parameter("outT", [C, BT], F32, isOutput=True)

    with tile.TileContext(nc) as tc:
        with (
            tc.sbuf_pool(name="statics", bufs=1) as statics,
            tc.sbuf_pool(name="pool_x", bufs=2) as pool_x,
            tc.sbuf_pool(name="pool_rope", bufs=1) as pool_rope,
            tc.sbuf_pool(name="pool_qk", bufs=1) as pool_qk,
            tc.sbuf_pool(name="pool_v", bufs=2) as pool_v,
            tc.sbuf_pool(name="pool_y", bufs=2) as pool_y,
            tc.sbuf_pool(name="pool_vs", bufs=2) as pool_vs,
            tc.sbuf_pool(name="pool_p", bufs=4) as pool_p,
            tc.sbuf_pool(name="pool_o", bufs=3) as pool_o,
            tc.sbuf_pool(name="pool_rb", bufs=2) as pool_rb,
            tc.psum_pool(name="ps_mm", bufs=3) as ps_mm,
            tc.psum_pool(name="ps_st", bufs=3) as ps_st,
            tc.psum_pool(name="ps_y", bufs=2) as ps_y,
        ):
            ident = statics.tile([128, 128], F32)
            make_identity(nc, ident)

            wqkv_sb = statics.tile([128, KC * 384], F32R)
            nc.sync.dma_start(out=wqkv_sb, in_=wqkv[:, :])
            wout_sb = statics.tile([128, C], F32R)
            nc.sync.dma_start(out=wout_sb, in_=wout[:, :])
            cos_sb = statics.tile([128, T], F32)
            nc.sync.dma_start(out=cos_sb, in_=cs[:, 0:T])
            sin_sb = statics.tile([128, T], F32)
            nc.sync.dma_start(out=sin_sb, in_=cs[:, T : 2 * T])

            def qkv_phase(b, qe, qo, ke, ko, v_all):
                # ones columns of v_all (64 per head per 256-block)
                vm = v_all.rearrange("p (m c) -> p m c", c=128)
                ones_bc = bass.AP(
                    tensor=ones, offset=0, ap=[[64, 128], [0, 2 * NKJ], [1, 64]]
                )
                nc.sync.dma_start(out=vm[:, :, 64:128], in_=ones_bc)

                for tci in range(4):
                    g = 4 * b + tci
                    xt_sb = pool_x.tile([128, KC, TC], F32R, tag="x", name=f"xt_{g}")
                    nc.sync.dma_start(
                        out=xt_sb,
                        in_=xt.rearrange("(kc p) n -> p kc n", p=128)[
                            :, :, g * TC : (g + 1) * TC
                        ],
                    )
                    pse = ps_mm.tile([128, TC], F32, tag="mm", name=f"pse_{g}")
                    pso = ps_mm.tile([128, TC], F32, tag="mm", name=f"pso_{g}")
                    psv = ps_mm.tile([128, TC], F32, tag="mm", name=f"psv_{g}")
                    for mi, ps in enumerate((pse, pso, psv)):
                        for kc in range(KC):
                            nc.tensor.matmul(
                                ps,
                                wqkv_sb[
                                    :, kc * 384 + 128 * mi : kc * 384 + 128 * (mi + 1)
                                ],
                                xt_sb[:, kc, :],
                                start=(kc == 0),
                                stop=(kc == KC - 1),
                            )
                    # RoPE
                    c_sl = cos_sb[:, tci * TC : (tci + 1) * TC]
                    s_sl = sin_sb[:, tci * TC : (tci + 1) * TC]
                    tEC = pool_rope.tile([128, TC], F32, tag="tEC", name=f"tEC_{g}")
                    nc.vector.tensor_mul(out=tEC, in0=pse, in1=c_sl)
                    tOS = pool_rope.tile([128, TC], F32, tag="tOS", name=f"tOS_{g}")
                    nc.vector.tensor_mul(out=tOS, in0=pso, in1=s_sl)
                    tES = pool_rope.tile([128, TC], F32, tag="tES", name=f"tES_{g}")
                    nc.vector.tensor_mul(out=tES, in0=pse, in1=s_sl)
                    tOC = pool_rope.tile([128, TC], F32, tag="tOC", name=f"tOC_{g}")
                    nc.vector.tensor_mul(out=tOC, in0=pso, in1=c_sl)

                    sl = slice(tci * TC, (tci + 1) * TC)
                    nc.vector.tensor_sub(out=qe[:, sl], in0=tEC[0:64], in1=tOS[0:64])
                    nc.vector.tensor_sub(out=ke[:, sl], in0=tEC[64:128], in1=tOS[64:128])
                    nc.vector.tensor_add(out=qo[:, sl], in0=tES[0:64], in1=tOC[0:64])
                    nc.vector.tensor_add(out=ko[:, sl], in0=tES[64:128], in1=tOC[64:128])

                    # v: PSUM -> SBUF, then transpose 128x128 blocks into v_all
                    v_sb = pool_vs.tile([128, TC], F32, tag="vs", name=f"vsb_{g}")
                    nc.scalar.activation(
                        out=v_sb, in_=psv, func=mybir.ActivationFunctionType.Copy
                    )
                    va = v_all.rearrange("p (n h c) -> p n h c", h=2, c=128)
                    for s in range(4):
                        j = 4 * tci + s
                        tr = ps_st.tile([128, 128], F32, tag="st", name=f"tr_{g}_{s}")
                        nc.tensor.transpose(
                            tr, v_sb[:, 128 * s : 128 * (s + 1)], ident
                        )
                        nc.vector.tensor_copy(
                            out=va[:, j, :, 0:64],
                            in_=tr.rearrange("p (h c) -> p h c", h=2),
                        )

            def attention_phase(b, qe, qo, ke, ko, v_all, y_t):
                for i in range(NQI):
                    nj = 4 * i + 4
                    for h in range(2):
                        yacc = ps_y.tile(
                            [128, TC], F32, tag="y", name=f"yacc_{b}_{i}_{h}"
                        )
                        hs = slice(32 * h, 32 * (h + 1))
                        for j in range(nj):
                            r = j - 4 * i
                            st = 128 * r if r > 0 else 0
                            ksl = slice(128 * j, 128 * (j + 1))
                            qsl = slice(TC * i + st, TC * (i + 1))
                            ps_s = ps_st.tile(
                                [128, TC], F32, tag="st", name=f"s_{b}_{i}_{h}_{j}"
                            )
                            nc.tensor.matmul(
                                ps_s[:, st:], ke[hs, ksl], qe[hs, qsl],
                                start=True, stop=False,
                            )
                            nc.tensor.matmul(
                                ps_s[:, st:], ko[hs, ksl], qo[hs, qsl],
                                start=False, stop=True,
                            )
                            p_sb = pool_p.tile(
                                [128, TC], F32R, tag="p", name=f"p_{b}_{i}_{h}_{j}"
                            )
                            nc.scalar.activation(
                                out=p_sb[:, st:], in_=ps_s[:, st:],
                                func=mybir.ActivationFunctionType.Exp, scale=0.125,
                            )
                            if r >= 0:
                                nc.gpsimd.affine_select(
                                    out=p_sb[:, st : st + 128],
                                    in_=p_sb[:, st : st + 128],
                                    pattern=[[1, 128]],
                                    channel_multiplier=-1,
                                    base=0,
                                    compare_op=mybir.AluOpType.is_ge,
                                    fill=0.0,
                                )
                            nc.tensor.matmul(
                                yacc[:, st:],
                                v_all[:, 256 * j + 128 * h : 256 * j + 128 * (h + 1)],
                                p_sb[:, st:],
                                start=(j == 0),
                                stop=(j == nj - 1),
                            )
                        rb = pool_rb.tile([64, TC], F32, tag="rb", name=f"rb_{b}_{i}_{h}")
                        nc.vector.reciprocal(out=rb, in_=yacc[64:128, :])
                        nc.vector.tensor_mul(
                            out=y_t[64 * h : 64 * (h + 1), TC * i : TC * (i + 1)],
                            in0=yacc[0:64, :],
                            in1=rb,
                        )

            def outproj_phase(b, y_t):
                for tci in range(4):
                    g = 4 * b + tci
                    for cc in range(KC):
                        ps = ps_mm.tile(
                            [128, TC], F32, tag="mm", name=f"op_{g}_{cc}"
                        )
                        nc.tensor.matmul(
                            ps,
                            wout_sb[:, 128 * cc : 128 * (cc + 1)],
                            y_t[:, TC * tci : TC * (tci + 1)],
                            start=True,
                            stop=True,
                        )
                        o_sb = pool_o.tile([128, TC], F32, tag="o", name=f"o_{g}_{cc}")
                        if cc % 2 == 0:
                            nc.scalar.activation(
                                out=o_sb, in_=ps,
                                func=mybir.ActivationFunctionType.Copy,
                            )
                        else:
                            nc.vector.tensor_copy(out=o_sb, in_=ps)
                        nc.sync.dma_start(
                            out=outT[
                                128 * cc : 128 * (cc + 1), g * TC : (g + 1) * TC
                            ],
                            in_=o_sb,
                        )

            state = {}
            for b in range(B):
                qe = pool_qk.tile([64, T], F32R, tag="qe", name=f"qe_{b}")
                qo = pool_qk.tile([64, T], F32R, tag="qo", name=f"qo_{b}")
                ke = pool_qk.tile([64, T], F32R, tag="ke", name=f"ke_{b}")
                ko = pool_qk.tile([64, T], F32R, tag="ko", name=f"ko_{b}")
                v_all = pool_v.tile([128, 256 * NKJ], F32R, tag="v", name=f"v_{b}")
                y_t = pool_y.tile([128, T], F32R, tag="y", name=f"y_{b}")
                state[b] = (qe, qo, ke, ko, v_all, y_t)

                qkv_phase(b, qe, qo, ke, ko, v_all)
                if b == 1:
                    outproj_phase(0, state[0][5])
                attention_phase(b, qe, qo, ke, ko, v_all, y_t)
            outproj_phase(1, state[1][5])

    nc.compile()
    return nc


_NC_CACHE = None


def _get_nc():
    global _NC_CACHE
    if _NC_CACHE is None:
        _NC_CACHE = build_nc()
    return _NC_CACHE


def _host_prep(x, qkv_w, out_w):
    x = np.asarray(x, dtype=np.float32)
    qkv_w = np.asarray(qkv_w, dtype=np.float32)
    out_w = np.asarray(out_w, dtype=np.float32)

    xt = np.ascontiguousarray(x.reshape(BT, C).T)  # [C, BT]

    # rope tables: row p uses frequency index p % 32
    t_idx = np.arange(T, dtype=np.float64)
    inv_freq = 1.0 / (10000.0 ** (np.arange(0, D, 2, dtype=np.float64) / D))  # 32
    ang = np.outer(np.tile(inv_freq, 4), t_idx)  # [128, T]
    cs = np.concatenate(
        [np.cos(ang), np.sin(ang)], axis=1
    ).astype(np.float32)  # [128, 2T]

    ones = np.ones((128, 64), np.float32)

    in_maps = []
    for core in range(N_CORES):
        h0 = HEADS_PER_CORE * core
        h1 = h0 + 1
        ev = np.arange(0, D, 2)
        od = np.arange(1, D, 2)
        e_rows = np.concatenate(
            [h0 * D + ev, h1 * D + ev, C + h0 * D + ev, C + h1 * D + ev]
        )
        o_rows = np.concatenate(
            [h0 * D + od, h1 * D + od, C + h0 * D + od, C + h1 * D + od]
        )
        v_rows = np.concatenate(
            [2 * C + h0 * D + np.arange(D), 2 * C + h1 * D + np.arange(D)]
        )
        rows = np.concatenate([e_rows, o_rows, v_rows])  # [384]
        w_part = qkv_w[rows]  # [384, C]
        # wqkv[p, kc*384 + m] = w_part[m, kc*128 + p]
        wqkv_c = np.ascontiguousarray(
            w_part.T.reshape(KC, 128, 384).transpose(1, 0, 2).reshape(128, KC * 384)
        )
        cols = np.concatenate([h0 * D + np.arange(D), h1 * D + np.arange(D)])
        wout_c = np.ascontiguousarray(out_w[:, cols].T)  # [128, C]
        in_maps.append(
            {"xt": xt, "wqkv": wqkv_c, "wout": wout_c, "cs": cs, "ones": ones}
        )
    return in_maps


def _run(in_maps, trace=False):
    nc = _get_nc()
    return run_bass_kernel_spmd(
        nc, in_maps, core_ids=list(range(N_CORES)), trace=trace
    )


def kernel(x, qkv_w, out_w, _trace=False, _results_box=None):
    in_maps = _host_prep(x, qkv_w, out_w)
    res = _run(in_maps, trace=_trace)
    if _results_box is not None:
        _results_box.append(res)
    acc = np.zeros((C, BT), np.float32)
    for r in res.results:
        acc += r["outT"]
    out = acc.T.reshape(B, T, C)
    return np.ascontiguousarray(out)
